# revision 11
# baseline (speedup 1.0000x reference)
"""2-layer GCN (GCNConv x2, symmetric norm, self-loops) on 8 Trainium2 NeuronCores.

v2 strategy (graph/data parallel):
  - Nodes partitioned contiguously across 8 cores (6250/core, padded to
    6272 = 49*128), permuted into 49 blocks of 128 with 2D greedy balancing
    of per-block in-degree split by src table half.
  - The gather table is split into two halves by BLOCK index (A = blocks
    0..24 of every core, B = blocks 25..48) so each layer's AllGather runs
    as two collectives that overlap compute (AG_a under phase A's tail,
    AG_b under the A-sweep of the aggregation).
  - norm = dinv[src]*dinv[dst] is separable: table rows store h' = dinv*h
    (folded into the PSUM->SBUF copy on the Scalar engine), and dinv[dst]
    is applied at aggregation output. The scatter matrix M is then a PURE
    one-hot, built in batched broadcast tensor_tensor is_equal ops (one per
    group x half) instead of per-chunk tensor_scalar -- this was the DVE
    bottleneck of v1.
  - Aggregation: per block, PSUM accumulates one-hot matmuls over the A
    sweep then the B sweep (software-pipelined with LOOKAHEAD groups so
    AG_b is hidden); finalize = scalar_tensor_tensor (dinv scale + bias) +
    activation.
  - dma_gather fetches h'[src] rows (512B L1 / 256B L2) from the DRAM
    tables; idx arrays are int16 (tables are 25600/24576 rows).

kernel(**inputs) takes full unsharded inputs, returns the full [50000, 128]
output. Self-contained: no sibling imports; /opt/trn_rl_repo provides bass.
"""

import math
import sys

import numpy as np

sys.path.insert(0, "/opt/trn_rl_repo")

import concourse.bass as bass  # noqa: E402
import concourse.mybir as mybir  # noqa: E402
import concourse.tile as tile  # noqa: E402
from concourse import bacc  # noqa: E402

P = 128
NCORES = 8
GG = 2         # blocks per gather group (msg tile granularity)
GMAX = 6       # chunks (of 128 idxs) per dma_gather op (HW cap 768)
GA = 8         # blocks per phase-A matmul group (one PSUM bank per block)
PAD_DST = 200.0  # mdst value for padding slots (never equals iota 0..127)

F32 = mybir.dt.float32
BF16 = mybir.dt.bfloat16
I16 = mybir.dt.int16


# ---------------------------------------------------------------------------
# host-side preprocessing
# ---------------------------------------------------------------------------

def _swizzle_idx(idx):
    """gather idx j -> [j%16, j//16], replicated across the 8 groups of 16."""
    n = idx.shape[0]
    a = np.zeros((16, n // 16), np.int16)
    a[np.arange(n) % 16, np.arange(n) // 16] = idx.astype(np.int16)
    return np.tile(a, (8, 1))


def _pack_blocks_2d(da, db, nb):
    """Greedy pack node ids into nb blocks of <=128 slots, balancing both
    half-loads (minimize max(load_a), max(load_b))."""
    n = da.shape[0]
    order = np.argsort(-(da + db), kind="stable")
    cur_a = np.zeros(nb)
    cur_b = np.zeros(nb)
    counts = np.zeros(nb, np.int64)
    pos = np.empty(n, np.int64)
    for i in order:
        score = np.maximum(cur_a + da[i], cur_b + db[i])
        score[counts >= P] = np.inf
        b = int(np.argmin(score))
        pos[i] = b * P + counts[b]
        counts[b] += 1
        cur_a[b] += da[i]
        cur_b[b] += db[i]
    return pos


def _prep(x, edge_index, W1, b1, W2, b2):
    import ml_dtypes

    N, F_in = x.shape
    F_h = W1.shape[1]
    F_out = W2.shape[1]
    assert N % NCORES == 0 and F_in % P == 0 and F_h == 2 * P and F_out == P
    npc_raw = N // NCORES
    nb = math.ceil(npc_raw / P)
    npc = nb * P
    na_blk = (nb + 1) // 2          # blocks in table A
    nbb_blk = nb - na_blk           # blocks in table B
    arows_pc = na_blk * P
    brows_pc = nbb_blk * P
    rows_a = NCORES * arows_pc
    rows_b = NCORES * brows_pc
    assert rows_a <= 2 ** 15 and rows_b <= 2 ** 15

    src = np.concatenate([np.asarray(edge_index[0]), np.arange(N)]).astype(np.int64)
    dst = np.concatenate([np.asarray(edge_index[1]), np.arange(N)]).astype(np.int64)
    deg = np.bincount(dst, minlength=N).astype(np.float64)
    dinv = np.where(deg > 0, 1.0 / np.sqrt(deg), 0.0)

    core_dst = dst // npc_raw

    # pass 1: pack by estimated half-degrees (proportional split)
    d_tot = np.bincount(dst, minlength=N).astype(np.float64)
    frac_a = arows_pc / npc
    pos = np.empty(N, np.int64)
    for c in range(NCORES):
        nodes = np.arange(c * npc_raw, (c + 1) * npc_raw)
        pos[nodes] = _pack_blocks_2d(d_tot[nodes] * frac_a,
                                     d_tot[nodes] * (1 - frac_a), nb)

    # pass 2: real half split from pass-1 positions, repack
    hi_b = pos[src] >= arows_pc
    d_a = np.bincount(dst[~hi_b], minlength=N).astype(np.float64)
    d_b = np.bincount(dst[hi_b], minlength=N).astype(np.float64)
    for c in range(NCORES):
        nodes = np.arange(c * npc_raw, (c + 1) * npc_raw)
        pos[nodes] = _pack_blocks_2d(d_a[nodes], d_b[nodes], nb)

    # final halves and per-(core, block, half) counts
    src_core = src // npc_raw
    src_pos = pos[src]
    in_b = src_pos >= arows_pc
    srow = np.where(in_b,
                    src_core * brows_pc + (src_pos - arows_pc),
                    src_core * arows_pc + src_pos)
    blk_of_dst = pos[dst] // P
    dl_of_dst = pos[dst] % P

    cnt = np.zeros((NCORES, nb, 2), np.int64)
    np.add.at(cnt, (core_dst, blk_of_dst, in_b.astype(np.int64)), 1)
    c_a = int(math.ceil(cnt[:, :, 0].max() / P))
    c_b = int(math.ceil(cnt[:, :, 1].max() / P))
    nch_a = nb * c_a
    nch_b = nb * c_b

    cores = []
    for c in range(NCORES):
        mask = core_dst == c
        e_srow = srow[mask]
        e_b = in_b[mask]
        e_blk = blk_of_dst[mask]
        e_dl = dl_of_dst[mask].astype(np.float64)

        idx_flat = {0: np.zeros(nch_a * P, np.int64),
                    1: np.zeros(nch_b * P, np.int64)}
        mdst_flat = {0: np.full(nch_a * P, PAD_DST, np.float64),
                     1: np.full(nch_b * P, PAD_DST, np.float64)}
        for h, c_h in ((0, c_a), (1, c_b)):
            sel = e_b == (h == 1)
            sr, bl, dl = e_srow[sel], e_blk[sel], e_dl[sel]
            order = np.argsort(bl, kind="stable")
            sr, bl, dl = sr[order], bl[order], dl[order]
            start = np.searchsorted(bl, np.arange(nb))
            end = np.searchsorted(bl, np.arange(nb) + 1)
            for b in range(nb):
                k = end[b] - start[b]
                assert k <= c_h * P
                o = b * c_h * P
                idx_flat[h][o:o + k] = sr[start[b]:end[b]]
                mdst_flat[h][o:o + k] = dl[start[b]:end[b]]

        # mdst swizzled to [P, nch]: value for slot (p, chunk)
        def slotwrap(v):
            nch = v.shape[0] // P
            return np.ascontiguousarray(
                v.reshape(nch, P).T).astype(ml_dtypes.bfloat16)

        mdst = np.concatenate([mdst_flat[0], mdst_flat[1]])

        # dinv per (slot, block)
        nodes = np.arange(c * npc_raw, (c + 1) * npc_raw)
        dinv_blk = np.zeros((P, nb), np.float32)
        pb = pos[nodes] // P
        pd = pos[nodes] % P
        dinv_blk[pd, pb] = dinv[nodes].astype(np.float32)
        dinv_rep = np.tile(dinv_blk.T.reshape(1, npc), (P, 1)).astype(
            ml_dtypes.bfloat16)

        # x slice, permuted and transposed: xt[f, pos] = x[node, f]
        xp = np.zeros((npc, F_in), np.float32)
        xp[pos[nodes]] = np.asarray(x[nodes], np.float32)
        xt = np.ascontiguousarray(xp.T).astype(ml_dtypes.bfloat16)

        cores.append({
            "xt": xt,
            "idx_a": _swizzle_idx(idx_flat[0]),
            "idx_b": _swizzle_idx(idx_flat[1]),
            "mdst": slotwrap(mdst),
            "dinv_blk": dinv_blk,
            "dinv_rep": dinv_rep,
        })

    shared = {
        "w1": np.asarray(W1, np.float32).astype(ml_dtypes.bfloat16),
        "w2": np.asarray(W2, np.float32).astype(ml_dtypes.bfloat16),
        "b1p": np.asarray(b1, np.float32).reshape(2, P).T.copy(),
        "b2b": np.tile(np.asarray(b2, np.float32)[None, :], (P, 1)),
        "iota": np.tile(
            np.arange(P, dtype=np.float32)[None, :], (P, 1)
        ).astype(ml_dtypes.bfloat16),
    }
    cfg = dict(N=N, F_in=F_in, F_h=F_h, F_out=F_out, npc_raw=npc_raw, nb=nb,
               npc=npc, na_blk=na_blk, nbb_blk=nbb_blk, arows_pc=arows_pc,
               brows_pc=brows_pc, rows_a=rows_a, rows_b=rows_b,
               c_a=c_a, c_b=c_b, nch_a=nch_a, nch_b=nch_b, pos=pos)
    return cfg, cores, shared


# ---------------------------------------------------------------------------
# device kernel
# ---------------------------------------------------------------------------

def _build_nc(cfg):
    F_in, F_h, F_out = cfg["F_in"], cfg["F_h"], cfg["F_out"]
    nb, npc = cfg["nb"], cfg["npc"]
    arows_pc, brows_pc = cfg["arows_pc"], cfg["brows_pc"]
    rows_a, rows_b = cfg["rows_a"], cfg["rows_b"]
    c_a, c_b = cfg["c_a"], cfg["c_b"]
    nch_a, nch_b = cfg["nch_a"], cfg["nch_b"]
    na_blk = cfg["na_blk"]
    kt = F_in // P
    rg = [list(range(NCORES))]
    groups = [list(range(s, min(s + GG, nb))) for s in range(0, nb, GG)]
    NG = len(groups)
    CM = GG * max(c_a, c_b)

    nc = bacc.Bacc(None, num_devices=NCORES, num_swdge_queues=4)

    xt_d = nc.declare_dram_parameter("xt", [F_in, npc], BF16, isOutput=False)
    w1_d = nc.declare_dram_parameter("w1", [F_in, F_h], BF16, isOutput=False)
    w2_d = nc.declare_dram_parameter("w2", [F_h, F_out], BF16, isOutput=False)
    b1_d = nc.declare_dram_parameter("b1p", [P, 2], F32, isOutput=False)
    b2_d = nc.declare_dram_parameter("b2b", [P, F_out], F32, isOutput=False)
    iota_d = nc.declare_dram_parameter("iota", [P, P], BF16, isOutput=False)
    ia_d = nc.declare_dram_parameter("idx_a", [P, nch_a * 8], I16, isOutput=False)
    ib_d = nc.declare_dram_parameter("idx_b", [P, nch_b * 8], I16, isOutput=False)
    mdst_d = nc.declare_dram_parameter("mdst", [P, nch_a + nch_b], BF16,
                                       isOutput=False)
    dblk_d = nc.declare_dram_parameter("dinv_blk", [P, nb], F32, isOutput=False)
    drep_d = nc.declare_dram_parameter("dinv_rep", [P, npc], BF16, isOutput=False)
    out_d = nc.declare_dram_parameter("out", [npc, F_out], F32, isOutput=True)

    qn = [0]

    with tile.TileContext(nc) as tc:
        with (
            tc.tile_pool(name="const", bufs=1) as const,
            tc.tile_pool(name="work", bufs=1) as work,
            tc.tile_pool(name="dram", bufs=1, space="DRAM") as dram,
        ):
            h_own = dram.tile([npc, F_h], BF16)
            h_full_a = dram.tile([rows_a, F_h], BF16, addr_space="Shared")
            h_full_b = dram.tile([rows_b, F_h], BF16, addr_space="Shared")
            g_own = dram.tile([npc, F_out], BF16)
            g_full_a = dram.tile([rows_a, F_out], BF16, addr_space="Shared")
            g_full_b = dram.tile([rows_b, F_out], BF16, addr_space="Shared")

            w1_t = const.tile([P, kt, F_h], BF16)
            w2_t = const.tile([P, 2, F_out], BF16)
            b1_t = const.tile([P, 2], F32)
            b2_t = const.tile([P, F_out], F32)
            iota_t = const.tile([P, P], BF16)
            ia_t = const.tile([P, nch_a * 8], I16)
            ib_t = const.tile([P, nch_b * 8], I16)
            mdst_t = const.tile([P, nch_a + nch_b], BF16)
            dblk_t = const.tile([P, nb], F32)
            drep_t = const.tile([P, npc], BF16)

            nc.sync.dma_start(w1_t[:], w1_d[:].rearrange("(a p) o -> p a o", p=P))
            nc.sync.dma_start(w2_t[:], w2_d[:].rearrange("(h p) o -> p h o", p=P))
            nc.sync.dma_start(b1_t[:], b1_d[:])
            nc.sync.dma_start(b2_t[:], b2_d[:])
            nc.sync.dma_start(iota_t[:], iota_d[:])
            nc.sync.dma_start(ia_t[:], ia_d[:])
            nc.sync.dma_start(ib_t[:], ib_d[:])
            nc.sync.dma_start(mdst_t[:], mdst_d[:])
            nc.sync.dma_start(dblk_t[:], dblk_d[:])
            nc.sync.dma_start(drep_t[:], drep_d[:])

            # ---- phase A: h' = dinv * (x @ W1), rows = this core's nodes ----
            xt_r = xt_d[:].rearrange("(a p) n -> p a n", p=P)
            psumA = tc.tile_pool(name="psumA", bufs=1, space="PSUM")
            psum = psumA.__enter__()
            ag_a_done = False
            for g0 in range(0, nb, GA):
                gb = list(range(g0, min(g0 + GA, nb)))
                phs = [psum.tile([P, F_h], F32, tag="ph", bufs=8,
                                 space="PSUM", name=f"ph{g0}_{i}")
                       for i in range(len(gb))]
                for a in range(kt):
                    xt_t = work.tile([P, GA * P], BF16, tag="xt", bufs=6)
                    nc.sync.dma_start(
                        xt_t[:, :len(gb) * P],
                        xt_r[:, a, g0 * P:g0 * P + len(gb) * P])
                    for i in range(len(gb)):
                        nc.tensor.matmul(phs[i][:],
                                         lhsT=xt_t[:, i * P:(i + 1) * P],
                                         rhs=w1_t[:, a, :],
                                         start=(a == 0), stop=(a == kt - 1))
                for i, b in enumerate(gb):
                    h_sb = work.tile([P, F_h], BF16, tag="hsb", bufs=4)
                    nc.scalar.activation(h_sb[:], phs[i][:],
                                         mybir.ActivationFunctionType.Copy,
                                         scale=dblk_t[:, b:b + 1])
                    nc.sync.dma_start(h_own[b * P:(b + 1) * P, :], h_sb[:])
                if (not ag_a_done) and gb[-1] >= na_blk - 1:
                    nc.gpsimd.collective_compute(
                        "AllGather", mybir.AluOpType.bypass, replica_groups=rg,
                        ins=[h_own[0:arows_pc, :]], outs=[h_full_a[:]])
                    ag_a_done = True
            nc.gpsimd.collective_compute(
                "AllGather", mybir.AluOpType.bypass, replica_groups=rg,
                ins=[h_own[arows_pc:npc, :]], outs=[h_full_b[:]])
            psumA.__exit__(None, None, None)

            # ---- helpers for aggregation sweeps ----
            def gathers(dstT, nch, table, idx_t, ch0, elem):
                for s in range(0, nch, GMAX):
                    k = min(GMAX, nch - s)
                    nc.gpsimd.dma_gather(
                        out_ap=dstT[:, s:s + k, :], in_ap=table,
                        idxs_ap=idx_t[:, (ch0 + s) * 8:(ch0 + s + k) * 8],
                        num_idxs=k * P, num_idxs_reg=k * P, elem_size=elem,
                        queue_num=qn[0] % 4)
                    qn[0] += 1

            def m_build(cg, moff):
                m_all = work.tile([P, CM, P], BF16, tag="mall", bufs=3)
                in0 = mdst_t[:, moff:moff + cg].rearrange(
                    "p (c one) -> p c one", one=1).to_broadcast([P, cg, P])
                in1 = iota_t[:].rearrange(
                    "p (one j) -> p one j", one=1).to_broadcast([P, cg, P])
                nc.vector.tensor_tensor(out=m_all[:, :cg, :], in0=in0, in1=in1,
                                        op=mybir.AluOpType.is_equal)
                return m_all

            # ---- phase C: aggregate layer 1, finalize, transform by W2 ----
            # Sweep A: accumulate each block's table-A chunks in PSUM, park
            # the partial in SBUF (Scalar-engine copy). Sweep B: accumulate
            # table-B chunks, then finalize (add stash, dinv scale, relu,
            # W2 transform). AG of h_full_b overlaps the whole sweep A.
            psumC = tc.tile_pool(name="psumC", bufs=1, space="PSUM")
            psum = psumC.__enter__()
            stash_c = {}

            for g in range(NG):
                blocks = groups[g]
                cg = len(blocks) * c_a
                msg = work.tile([P, CM, F_h], BF16, tag="msgC", bufs=3)
                gathers(msg, cg, h_full_a[:], ia_t, blocks[0] * c_a, F_h)
                m_all = m_build(cg, blocks[0] * c_a)
                for i, b in enumerate(blocks):
                    pah = [psum.tile([P, P], F32, tag="pah", bufs=4,
                                     space="PSUM", name=f"paA{b}_{_h}")
                           for _h in range(2)]
                    for ci in range(c_a):
                        cc = i * c_a + ci
                        for h in range(2):
                            nc.tensor.matmul(pah[h][:],
                                             lhsT=msg[:, cc, h * P:(h + 1) * P],
                                             rhs=m_all[:, cc, :],
                                             start=(ci == 0),
                                             stop=(ci == c_a - 1))
                    st = work.tile([P, 2, P], F32, tag="stashC", bufs=nb)
                    for h in range(2):
                        nc.scalar.activation(st[:, h, :], pah[h][:],
                                             mybir.ActivationFunctionType.Copy)
                    stash_c[b] = st

            ag_ga_done = False
            for g in range(NG):
                blocks = groups[g]
                cg = len(blocks) * c_b
                msg = work.tile([P, CM, F_h], BF16, tag="msgC", bufs=3)
                gathers(msg, cg, h_full_b[:], ib_t, blocks[0] * c_b, F_h)
                m_all = m_build(cg, nch_a + blocks[0] * c_b)
                for i, b in enumerate(blocks):
                    pah = [psum.tile([P, P], F32, tag="pah", bufs=4,
                                     space="PSUM", name=f"paB{b}_{_h}")
                           for _h in range(2)]
                    for ci in range(c_b):
                        cc = i * c_b + ci
                        for h in range(2):
                            nc.tensor.matmul(pah[h][:],
                                             lhsT=msg[:, cc, h * P:(h + 1) * P],
                                             rhs=m_all[:, cc, :],
                                             start=(ci == 0),
                                             stop=(ci == c_b - 1))
                    st = stash_c.pop(b)
                    t = work.tile([P, F_h], F32, tag="finC", bufs=3)
                    ra = work.tile([P, 2, P], BF16, tag="ra", bufs=3)
                    for h in range(2):
                        nc.vector.scalar_tensor_tensor(
                            out=t[:, h * P:(h + 1) * P], in0=pah[h][:],
                            scalar=0.0, in1=st[:, h, :],
                            op0=mybir.AluOpType.bypass,
                            op1=mybir.AluOpType.add)
                        nc.vector.scalar_tensor_tensor(
                            out=t[:, h * P:(h + 1) * P],
                            in0=t[:, h * P:(h + 1) * P], scalar=0.0,
                            in1=drep_t[:, b * P:(b + 1) * P],
                            op0=mybir.AluOpType.bypass,
                            op1=mybir.AluOpType.mult)
                        nc.scalar.activation(ra[:, h, :], t[:, h * P:(h + 1) * P],
                                             mybir.ActivationFunctionType.Relu,
                                             bias=b1_t[:, h:h + 1], scale=1.0)
                    pg = psum.tile([P, F_out], F32, tag="pg", bufs=2,
                                   space="PSUM")
                    nc.tensor.matmul(pg[:], lhsT=ra[:, 0, :], rhs=w2_t[:, 0, :],
                                     start=True, stop=False)
                    nc.tensor.matmul(pg[:], lhsT=ra[:, 1, :], rhs=w2_t[:, 1, :],
                                     start=False, stop=True)
                    g_sb = work.tile([P, F_out], BF16, tag="gsb", bufs=3)
                    nc.scalar.activation(g_sb[:], pg[:],
                                         mybir.ActivationFunctionType.Copy,
                                         scale=dblk_t[:, b:b + 1])
                    nc.sync.dma_start(g_own[b * P:(b + 1) * P, :], g_sb[:])
                    if (not ag_ga_done) and b >= na_blk - 1:
                        nc.gpsimd.collective_compute(
                            "AllGather", mybir.AluOpType.bypass,
                            replica_groups=rg,
                            ins=[g_own[0:arows_pc, :]], outs=[g_full_a[:]])
                        ag_ga_done = True
            nc.gpsimd.collective_compute(
                "AllGather", mybir.AluOpType.bypass, replica_groups=rg,
                ins=[g_own[arows_pc:npc, :]], outs=[g_full_b[:]])
            psumC.__exit__(None, None, None)

            # ---- phase E: aggregate layer 2, scale + bias, write out ----
            psumE = tc.tile_pool(name="psumE", bufs=1, space="PSUM")
            psum = psumE.__enter__()
            stash_e = {}

            for g in range(NG):
                blocks = groups[g]
                cg = len(blocks) * c_a
                msg = work.tile([P, CM, F_out], BF16, tag="msgE", bufs=3)
                gathers(msg, cg, g_full_a[:], ia_t, blocks[0] * c_a, F_out)
                m_all = m_build(cg, blocks[0] * c_a)
                for i, b in enumerate(blocks):
                    po = psum.tile([P, F_out], F32, tag="po", bufs=6,
                                   space="PSUM")
                    for ci in range(c_a):
                        cc = i * c_a + ci
                        nc.tensor.matmul(po[:], lhsT=m_all[:, cc, :],
                                         rhs=msg[:, cc, :], start=(ci == 0),
                                         stop=(ci == c_a - 1))
                    st = work.tile([P, F_out], F32, tag="stashE", bufs=nb)
                    nc.scalar.activation(st[:], po[:],
                                         mybir.ActivationFunctionType.Copy)
                    stash_e[b] = st

            for g in range(NG):
                blocks = groups[g]
                cg = len(blocks) * c_b
                msg = work.tile([P, CM, F_out], BF16, tag="msgE", bufs=3)
                gathers(msg, cg, g_full_b[:], ib_t, blocks[0] * c_b, F_out)
                m_all = m_build(cg, nch_a + blocks[0] * c_b)
                for i, b in enumerate(blocks):
                    po = psum.tile([P, F_out], F32, tag="po", bufs=6,
                                   space="PSUM")
                    for ci in range(c_b):
                        cc = i * c_b + ci
                        nc.tensor.matmul(po[:], lhsT=m_all[:, cc, :],
                                         rhs=msg[:, cc, :], start=(ci == 0),
                                         stop=(ci == c_b - 1))
                    st = stash_e.pop(b)
                    t = work.tile([P, F_out], F32, tag="finE", bufs=3)
                    nc.vector.scalar_tensor_tensor(
                        out=t[:], in0=po[:], scalar=0.0, in1=st[:],
                        op0=mybir.AluOpType.bypass, op1=mybir.AluOpType.add)
                    o_sb = work.tile([P, F_out], F32, tag="osb", bufs=3)
                    nc.vector.scalar_tensor_tensor(
                        out=o_sb[:], in0=t[:], scalar=dblk_t[:, b:b + 1],
                        in1=b2_t[:], op0=mybir.AluOpType.mult,
                        op1=mybir.AluOpType.add)
                    nc.sync.dma_start(out_d[b * P:(b + 1) * P, :], o_sb[:])
            psumE.__exit__(None, None, None)

    nc.compile()
    return nc


def _in_maps(cfg, cores, shared):
    return [{**shared, **c} for c in cores]


def _assemble(cfg, outs):
    N, F_out, npc_raw = cfg["N"], cfg["F_out"], cfg["npc_raw"]
    pos = cfg["pos"]
    full = np.empty((N, F_out), np.float32)
    for c in range(NCORES):
        nodes = np.arange(c * npc_raw, (c + 1) * npc_raw)
        full[nodes] = outs[c][pos[nodes]]
    return full


# ---------------------------------------------------------------------------
# entry points
# ---------------------------------------------------------------------------

def kernel(x, edge_index, W1, b1, W2, b2):
    cfg, cores, shared = _prep(x, edge_index, W1, b1, W2, b2)
    nc = _build_nc(cfg)
    from concourse.bass_utils import run_bass_kernel_spmd
    res = run_bass_kernel_spmd(nc, _in_maps(cfg, cores, shared),
                               list(range(NCORES)))
    return _assemble(cfg, [r["out"] for r in res.results])


def run_profiled(x, edge_index, W1, b1, W2, b2, tmpdir=None):
    """Like kernel(), but traces on HW; returns (out, exec_time_ns, tmpdir)."""
    import time

    t0 = time.time()
    cfg, cores, shared = _prep(x, edge_index, W1, b1, W2, b2)
    print(f"prep {time.time() - t0:.1f}s; cfg c_a={cfg['c_a']} "
          f"c_b={cfg['c_b']} nb={cfg['nb']}")
    t0 = time.time()
    nc = _build_nc(cfg)
    print(f"build {time.time() - t0:.1f}s; {len(nc.inst_map)} instructions")
    from concourse.bass_utils import run_bass_kernel_spmd
    in_maps = _in_maps(cfg, cores, shared)
    t0 = time.time()
    res = run_bass_kernel_spmd(nc, in_maps, list(range(NCORES)))
    print(f"run {time.time() - t0:.1f}s")
    out = _assemble(cfg, [r["out"] for r in res.results])
    exec_ns = None
    try:
        t0 = time.time()
        res2 = run_bass_kernel_spmd(nc, in_maps, list(range(NCORES)),
                                    trace=True, tmpdir=tmpdir)
        print(f"traced run {time.time() - t0:.1f}s")
        exec_ns = res2.exec_time_ns
    except Exception as e:
        print(f"trace run failed: {type(e).__name__}: {str(e)[:200]}")
    return out, exec_ns, tmpdir


def _numpy_ref(x, edge_index, W1, b1, W2, b2):
    N = x.shape[0]
    src = np.concatenate([edge_index[0], np.arange(N)])
    dst = np.concatenate([edge_index[1], np.arange(N)])
    deg = np.bincount(dst, minlength=N).astype(np.float64)
    dinv = np.where(deg > 0, 1 / np.sqrt(deg), 0)
    nrm = (dinv[src] * dinv[dst]).astype(np.float32)

    def layer(h, W, b):
        hw = h @ W
        out = np.zeros((N, W.shape[1]), np.float32)
        np.add.at(out, dst, hw[src] * nrm[:, None])
        return out + b

    h = np.maximum(layer(x, W1, b1), 0)
    return layer(h, W2, b2)


def _selftest_sim():
    from concourse import bass_interp
    rng = np.random.default_rng(1)
    N, E, F_in = 2048, 8192, 512
    x = rng.standard_normal((N, F_in), dtype=np.float32)
    ei = rng.integers(0, N, (2, E)).astype(np.int64)
    W1 = (rng.standard_normal((F_in, 256), dtype=np.float32) * F_in ** -0.5)
    W2 = (rng.standard_normal((256, 128), dtype=np.float32) * 256 ** -0.5)
    b1 = rng.standard_normal(256).astype(np.float32) * 0.1
    b2 = rng.standard_normal(128).astype(np.float32) * 0.1

    cfg, cores, shared = _prep(x, ei, W1, b1, W2, b2)
    print("cfg:", {k: v for k, v in cfg.items() if k != "pos"})
    nc = _build_nc(cfg)
    print("built; instructions:", len(nc.inst_map))

    sim = bass_interp.MultiCoreSim(nc, NCORES)
    for i, m in enumerate(_in_maps(cfg, cores, shared)):
        for k, v in m.items():
            sim.cores[i].tensor(k)[:] = v
    sim.simulate()
    outs = [np.array(sim.cores[i].mem_tensor("out")) for i in range(NCORES)]
    got = _assemble(cfg, outs)
    want = _numpy_ref(x, ei, W1, b1, W2, b2)
    err = np.abs(got - want).max() / (np.abs(want).max() + 1e-9)
    print("selftest rel err:", err)
    assert err < 1e-2, "selftest FAILED"
    print("SELFTEST PASSED")


if __name__ == "__main__":
    _selftest_sim()
